# revision 8
# baseline (speedup 1.0000x reference)
"""2-layer GCN (GCNEncoder) on 8 Trainium2 NeuronCores via Bass.

Strategy (1D node partitioning, dst-major):
- Nodes are split evenly across 8 cores (12500 each, padded to 12544 slots).
- Within a core, nodes are sorted by in-degree (desc) so 128-node tiles have
  near-uniform padded widths K_t; each node's in-edges (+ its self-loop) are
  padded to K_t slots.
- Algebraic reshaping:  A@(x@W) == (A@x)@W, so both convs aggregate 16-wide
  features:   out = dinv * segsum(w * xs[src]) ;  xs = dinv * x.
- The per-edge gather runs on the DMA engines via the dma_gather ucode
  (int16 indices -> table packed 4 nodes per 256B row; selection of the
  right quarter is folded into host-expanded weights wj = w * onehot4).
- deg/dinv are computed on-device from the padded weights; dinv and the
  inter-layer activations are exchanged with AllGather collectives.
"""
import sys
sys.path.insert(0, "/opt/trn_rl_repo")

import numpy as np

N_NODES = 100000
N_CORES = 8
NL = 12500            # nodes per core
P = 128
NT = 98               # tiles per core (98*128 = 12544 slots)
SLOTS = NT * P        # 12544
N_TAB = N_CORES * SLOTS   # 100352 table rows
N_GRP = N_TAB // 4        # 25088 packed 4-node groups (int16-safe indices)
IN_CH = 16
HIDDEN = 128
OUT_CH = 16
MAX_IDX_PER_CALL = 8192   # dma_gather single_packet=False validated limit


# ----------------------------------------------------------------------------
# host-side graph preprocessing (index manipulation / sharding only)
# ----------------------------------------------------------------------------

def _prep_graph(edge_index, edge_weight):
    src = np.asarray(edge_index[0], dtype=np.int64)
    dst = np.asarray(edge_index[1], dtype=np.int64)
    w = np.asarray(edge_weight, dtype=np.float32)

    core_of = dst // NL          # owner core of each edge (by dst)
    # counts per node (in-degree + self loop)
    cnt = np.bincount(dst, minlength=N_NODES).astype(np.int64) + 1

    # per-core degree-sorted local ordering (stable for determinism)
    order = np.empty(N_NODES, dtype=np.int64)    # order[slot_global] = orig node
    slot_of = np.empty(N_NODES, dtype=np.int64)  # slot_of[orig] = global slot
    K_t = np.zeros(NT, dtype=np.int64)
    for r in range(N_CORES):
        nodes = np.arange(r * NL, (r + 1) * NL)
        loc_order = nodes[np.argsort(-cnt[nodes], kind="stable")]
        gs = r * SLOTS + np.arange(NL)
        order_r = np.full(SLOTS, -1, dtype=np.int64)
        order_r[:NL] = loc_order
        slot_of[loc_order] = gs
        if r == 0:
            order_full = np.full(N_TAB, -1, dtype=np.int64)
        order_full[r * SLOTS:(r + 1) * SLOTS] = order_r
        # per-tile max count for this core
        c = np.zeros(SLOTS, dtype=np.int64)
        c[:NL] = cnt[loc_order]
        c[NL:] = 1  # dummies get a self-loop
        K_t = np.maximum(K_t, c.reshape(NT, P).max(axis=1))
    order = order_full

    # remap edge endpoints into slot space
    src_s = slot_of[src]
    dst_s = slot_of[dst]

    # build padded slot arrays per core
    K_t = np.maximum(K_t, 1)
    # split any tile wider than MAX_IDX_PER_CALL/128 at gather time (below)
    tile_off = np.concatenate([[0], np.cumsum(K_t)])
    W_total = int(tile_off[-1])          # total K across tiles (per core)

    idx16_cores = []
    wj_cores = []
    for r in range(N_CORES):
        gsrc = np.zeros((P, W_total), dtype=np.int64)   # global slot of src
        wpad = np.zeros((P, W_total), dtype=np.float32)
        # self loops for every slot (incl. dummies): own slot, weight 1
        own = r * SLOTS + (np.arange(SLOTS).reshape(NT, P))
        fill = np.zeros((NT, P), dtype=np.int64)
        for t in range(NT):
            gsrc[:, tile_off[t]] = own[t]
            wpad[:, tile_off[t]] = 1.0
        fill[:, :] = 1
        # real edges of this core
        m = core_of == r
        es, ed, ew = src_s[m], dst_s[m], w[m]
        ls = ed - r * SLOTS       # local slot 0..12543
        et = ls // P              # tile
        ep = ls % P               # partition
        # assign k-position per edge via running fill counters
        ordm = np.argsort(ls, kind="stable")
        es, ew, et, ep, ls = es[ordm], ew[ordm], et[ordm], ep[ordm], ls[ordm]
        # position within its node's list:
        node_start = np.searchsorted(ls, np.arange(SLOTS), side="left")
        node_end = np.searchsorted(ls, np.arange(SLOTS), side="right")
        kpos = np.arange(len(ls)) - node_start[ls] + 1  # +1 after self loop
        col = tile_off[et] + kpos
        gsrc[ep, col] = es
        wpad[ep, col] = ew
        # pack: group + phase
        grp = (gsrc // 4).astype(np.int16)
        ph = (gsrc % 4).astype(np.int64)
        wj = np.zeros((P, W_total, 4), dtype=np.float32)
        wj[np.arange(P)[:, None], np.arange(W_total)[None, :], ph] = wpad
        # idx stream per tile: i = k*128 + p  ->  int16 [16, 8*K_t] per tile
        idx16 = np.empty((16, W_total * 8), dtype=np.int16)
        for t in range(NT):
            k0, k1 = tile_off[t], tile_off[t + 1]
            stream = grp[:, k0:k1].T.reshape(-1)          # [K_t*128] k-major
            blk = stream.reshape(-1, 16).T                # [16, 8*K_t]
            idx16[:, 8 * k0:8 * k1] = blk
        idx16_cores.append(idx16)
        wj_cores.append(wj.reshape(P, W_total * 4))

    return order, slot_of, K_t, tile_off, W_total, idx16_cores, wj_cores


# ----------------------------------------------------------------------------
# bass program
# ----------------------------------------------------------------------------

def _build_program(K_t, tile_off, W_total):
    import concourse.bass as bass
    import concourse.bacc as bacc
    import concourse.mybir as mybir
    import concourse.tile as tile
    from concourse.masks import make_identity

    f32 = mybir.dt.float32
    nc = bacc.Bacc(None, num_devices=N_CORES)

    xp = nc.dram_tensor("xp", [N_TAB, IN_CH], f32, kind="ExternalInput")
    idxs = nc.dram_tensor("idxs", [16, W_total * 8], mybir.dt.int16,
                          kind="ExternalInput")
    wj = nc.dram_tensor("wj", [P, W_total * 4], f32, kind="ExternalInput")
    w1 = nc.dram_tensor("w1", [IN_CH, HIDDEN], f32, kind="ExternalInput")
    b1 = nc.dram_tensor("b1", [HIDDEN], f32, kind="ExternalInput")
    w2 = nc.dram_tensor("w2", [HIDDEN, OUT_CH], f32, kind="ExternalInput")
    b2 = nc.dram_tensor("b2", [OUT_CH], f32, kind="ExternalInput")
    out = nc.dram_tensor("out", [SLOTS, OUT_CH], f32, kind="ExternalOutput")

    xs_dram = nc.dram_tensor("xs_dram", [N_TAB, IN_CH], f32)
    dloc = nc.dram_tensor("dloc", [SLOTS], f32)
    dfull = nc.dram_tensor("dfull", [N_TAB], f32)
    zloc = nc.dram_tensor("zloc", [SLOTS, OUT_CH], f32)
    zfull = nc.dram_tensor("zfull", [N_TAB, OUT_CH], f32, addr_space="Shared")

    # gather-call split: tiles wider than MAX_IDX/128 split along k
    def gather_pieces(t):
        k0, k1 = int(tile_off[t]), int(tile_off[t + 1])
        kmax = MAX_IDX_PER_CALL // P
        pieces = []
        k = k0
        while k < k1:
            ke = min(k + kmax, k1)
            pieces.append((k, ke))
            k = ke
        return pieces

    with tile.TileContext(nc) as tc:
        with (
            tc.tile_pool(name="const", bufs=1) as cpool,
            tc.tile_pool(name="io", bufs=3) as iopool,
            tc.tile_pool(name="gat", bufs=3) as gpool,
            tc.tile_pool(name="met", bufs=4) as mpool,
            tc.tile_pool(name="big", bufs=1) as bigpool,
            tc.tile_pool(name="ps", bufs=2, space="PSUM") as pspool,
            tc.tile_pool(name="ps2", bufs=2, space="PSUM") as ps2pool,
        ):
            ident = cpool.tile([P, P], f32)
            make_identity(nc, ident[:])
            w1_sb = cpool.tile([IN_CH, HIDDEN], f32)
            nc.sync.dma_start(out=w1_sb[:], in_=w1[:])
            b1_sb = cpool.tile([HIDDEN, 1], f32)
            nc.sync.dma_start(out=b1_sb[:], in_=b1[:, None])
            w2_sb = cpool.tile([HIDDEN, OUT_CH], f32)
            nc.sync.dma_start(out=w2_sb[:], in_=w2[:])
            b2_rep = cpool.tile([P, OUT_CH], f32)
            nc.sync.dma_start(out=b2_rep[:],
                              in_=b2[None, :].broadcast_to([P, OUT_CH]))

            # wj resident (needed for deg + both layers)
            wj_sb = bigpool.tile([P, W_total * 4], f32)
            nc.sync.dma_start(out=wj_sb[:], in_=wj[:])

            # ---- deg / dinv ----
            deg_sb = cpool.tile([P, NT], f32)
            for t in range(NT):
                k0, k1 = int(tile_off[t]), int(tile_off[t + 1])
                nc.vector.tensor_reduce(
                    out=deg_sb[:, t:t + 1], in_=wj_sb[:, 4 * k0:4 * k1],
                    axis=mybir.AxisListType.X, op=mybir.AluOpType.add)
            sq_sb = cpool.tile([P, NT], f32)
            nc.scalar.activation(out=sq_sb[:], in_=deg_sb[:],
                                 func=mybir.ActivationFunctionType.Sqrt)
            dinv_sb = cpool.tile([P, NT], f32)
            nc.vector.reciprocal(out=dinv_sb[:], in_=sq_sb[:])
            # dloc in slot order: slot = t*128 + p
            nc.sync.dma_start(out=dloc[:].rearrange("(t p) -> p t", p=P),
                              in_=dinv_sb[:])
            nc.gpsimd.collective_compute(
                "AllGather", mybir.AluOpType.bypass,
                replica_groups=[list(range(N_CORES))],
                ins=[dloc[:]], outs=[dfull[:]])

            # ---- xs = x * dinv (full table), written to DRAM ----
            NCHUNK = 16
            rows_per = N_TAB // NCHUNK          # 6272
            it_per = rows_per // P              # 49
            for c in range(NCHUNK):
                r0 = c * rows_per
                xc = iopool.tile([P, it_per * IN_CH], f32, name="xc", tag="xc")
                nc.sync.dma_start(
                    out=xc[:],
                    in_=xp[r0:r0 + rows_per, :].rearrange("(i p) c -> p i c", p=P))
                dc = iopool.tile([P, it_per], f32, name="dc", tag="dc")
                nc.sync.dma_start(
                    out=dc[:],
                    in_=dfull[r0:r0 + rows_per].rearrange("(i p) -> p i", p=P))
                xcv = xc[:].rearrange("p (i c) -> p i c", c=IN_CH)
                nc.vector.tensor_tensor(
                    out=xcv, in0=xcv,
                    in1=dc[:].unsqueeze(-1).broadcast_to([P, it_per, IN_CH]),
                    op=mybir.AluOpType.mult)
                nc.sync.dma_start(
                    out=xs_dram[r0:r0 + rows_per, :].rearrange(
                        "(i p) c -> p i c", p=P),
                    in_=xc[:])

            out1T = bigpool.tile([P, SLOTS], f32)   # relu(g1@W1+b1), ch-major

            # ---- layer aggregation pipeline (shared) ----
            def aggregate(t, table_view):
                """returns r_t tile [P, 16] = sum_k w*table[src] for tile t."""
                k0, k1 = int(tile_off[t]), int(tile_off[t + 1])
                Kt = k1 - k0
                idx_t = gpool.tile([P, 8 * (MAX_IDX_PER_CALL // P)],
                                   mybir.dt.int16, name="idx_t", tag="idx_t")
                nc.sync.dma_start(
                    out=idx_t[:, :8 * Kt],
                    in_=idxs[:, 8 * k0:8 * k1].unsqueeze(0).broadcast_to(
                        [8, 16, 8 * Kt]))
                G = gpool.tile([P, (MAX_IDX_PER_CALL // P) * 64], f32,
                               name="G", tag="G")
                for (ka, kb) in gather_pieces(t):
                    n_idx = (kb - ka) * P
                    nc.gpsimd.dma_gather(
                        out_ap=G[:, (ka - k0) * 64:(kb - k0) * 64].rearrange(
                            "p (k e) -> p k e", e=64),
                        in_ap=table_view,
                        idxs_ap=idx_t[:, 8 * (ka - k0):8 * (kb - k0)],
                        num_idxs=n_idx,
                        num_idxs_reg=n_idx,
                        elem_size=64,
                        elem_step=64,
                        single_packet=False,
                    )
                Gv = G[:, :Kt * 64].rearrange("p (k c) -> p k c", c=IN_CH)
                nc.vector.tensor_tensor(
                    out=Gv, in0=Gv,
                    in1=wj_sb[:, 4 * k0:4 * k1].unsqueeze(-1).broadcast_to(
                        [P, 4 * Kt, IN_CH]),
                    op=mybir.AluOpType.mult)
                r_t = mpool.tile([P, IN_CH], f32, name="r_t", tag="r_t")
                nc.vector.tensor_reduce(
                    out=r_t[:],
                    in_=G[:, :Kt * 64].rearrange("p (k c) -> p c k", c=IN_CH),
                    axis=mybir.AxisListType.X, op=mybir.AluOpType.add)
                return r_t

            xs_view = xs_dram[:].rearrange("(a b) c -> a (b c)", b=4)

            # ---- layer 1 ----
            for t in range(NT):
                r_t = aggregate(t, xs_view)
                g1s = mpool.tile([P, IN_CH], f32, name="g1s", tag="g1s")
                nc.vector.tensor_scalar_mul(out=g1s[:], in0=r_t[:],
                                            scalar1=dinv_sb[:, t:t + 1])
                g1T_ps = pspool.tile([IN_CH, P], f32, space="PSUM",
                                     name="g1T_ps", tag="g1T_ps")
                nc.tensor.transpose(out=g1T_ps[:], in_=g1s[:], identity=ident[:])
                g1T = mpool.tile([IN_CH, P], f32, name="g1T", tag="g1T")
                nc.vector.tensor_copy(out=g1T[:], in_=g1T_ps[:])
                h_ps = ps2pool.tile([P, P], f32, space="PSUM",
                                    name="h_ps", tag="h_ps")
                nc.tensor.matmul(out=h_ps[:], lhsT=w1_sb[:], rhs=g1T[:],
                                 start=True, stop=True)
                nc.scalar.activation(out=out1T[:, t * P:(t + 1) * P], in_=h_ps[:],
                                     func=mybir.ActivationFunctionType.Relu,
                                     bias=b1_sb[:])

            # ---- z = out1 @ W2, zs = dinv*z  -> zloc -> AllGather ----
            zloc_sb = bigpool.tile([P, NT * OUT_CH], f32)
            CH = 512
            for c0 in range(0, SLOTS, CH):
                ce = min(c0 + CH, SLOTS)
                cw = ce - c0
                z_ps = ps2pool.tile([OUT_CH, CH], f32, space="PSUM",
                                    name="z_ps", tag="z_ps")
                nc.tensor.matmul(out=z_ps[:, :cw], lhsT=w2_sb[:],
                                 rhs=out1T[:, c0:ce], start=True, stop=True)
                zch = mpool.tile([OUT_CH, CH], f32, name="zch", tag="zch")
                nc.vector.tensor_copy(out=zch[:, :cw], in_=z_ps[:, :cw])
                for j in range(cw // P):
                    t = (c0 + j * P) // P
                    ztr_ps = pspool.tile([P, OUT_CH], f32, space="PSUM",
                                         name="ztr_ps", tag="ztr_ps")
                    nc.tensor.transpose(out=ztr_ps[:],
                                        in_=zch[:, j * P:(j + 1) * P],
                                        identity=ident[0:OUT_CH, 0:OUT_CH])
                    nc.vector.tensor_scalar_mul(
                        out=zloc_sb[:, t * OUT_CH:(t + 1) * OUT_CH],
                        in0=ztr_ps[:], scalar1=dinv_sb[:, t:t + 1])
            nc.sync.dma_start(
                out=zloc[:].rearrange("(t p) c -> p t c", p=P),
                in_=zloc_sb[:])
            nc.gpsimd.collective_compute(
                "AllGather", mybir.AluOpType.bypass,
                replica_groups=[list(range(N_CORES))],
                ins=[zloc[:]], outs=[zfull[:]])

            zs_view = zfull[:].rearrange("(a b) c -> a (b c)", b=4)

            # ---- layer 2 ----
            out_sb = bigpool.tile([P, NT * OUT_CH], f32)
            for t in range(NT):
                r_t = aggregate(t, zs_view)
                o_t = mpool.tile([P, OUT_CH], f32, name="o_t", tag="o_t")
                nc.vector.tensor_scalar_mul(out=o_t[:], in0=r_t[:],
                                            scalar1=dinv_sb[:, t:t + 1])
                nc.vector.tensor_tensor(
                    out=out_sb[:, t * OUT_CH:(t + 1) * OUT_CH],
                    in0=o_t[:], in1=b2_rep[:], op=mybir.AluOpType.add)
            nc.sync.dma_start(
                out=out[:].rearrange("(t p) c -> p t c", p=P),
                in_=out_sb[:])

    nc.compile()
    return nc


_CACHE = {}


def kernel(x, edge_index, edge_weight, W1, b1, W2, b2):
    x = np.asarray(x, dtype=np.float32)
    W1 = np.asarray(W1, dtype=np.float32)
    b1 = np.asarray(b1, dtype=np.float32)
    W2 = np.asarray(W2, dtype=np.float32)
    b2 = np.asarray(b2, dtype=np.float32)

    (order, slot_of, K_t, tile_off, W_total,
     idx16_cores, wj_cores) = _prep_graph(edge_index, edge_weight)

    # permuted/padded features: row g -> x[order[g]] (zeros for dummies)
    xp = np.zeros((N_TAB, IN_CH), dtype=np.float32)
    valid = order >= 0
    xp[valid] = x[order[valid]]

    key = (int(W_total), tuple(int(k) for k in K_t))
    if key not in _CACHE:
        _CACHE[key] = _build_program(K_t, tile_off, W_total)
    nc = _CACHE[key]

    in_maps = []
    for r in range(N_CORES):
        in_maps.append(dict(
            xp=xp, idxs=idx16_cores[r], wj=wj_cores[r],
            w1=W1, b1=b1, w2=W2, b2=b2,
        ))

    global _LAST_IN_MAPS
    _LAST_IN_MAPS = in_maps
    from concourse.bass_utils import run_bass_kernel_spmd
    res = run_bass_kernel_spmd(nc, in_maps, core_ids=list(range(N_CORES)))

    out_full = np.empty((N_NODES, OUT_CH), dtype=np.float32)
    for r in range(N_CORES):
        o = res.results[r]["out"]          # [SLOTS, 16] in slot order
        seg = order[r * SLOTS:(r + 1) * SLOTS]
        v = seg >= 0
        out_full[seg[v]] = o[v]
    return out_full


if __name__ == "__main__":
    import reference
    inputs = reference.setup_inputs()
    inputs = {k: np.asarray(v) for k, v in inputs.items()}
    got = kernel(**inputs)
    exp = np.asarray(reference.reference(**inputs))
    err = np.abs(got - exp).max() / (np.abs(exp).max() + 1e-30)
    print("Relative error:", err)
